# revision 4
# baseline (speedup 1.0000x reference)
"""Trainium2 Bass kernel v2 for nn_AuxiliaryLoss (FAPE + torsion loss).

Math: d2(i,j) = <F_i, Z_j> rank-49 quadratic form (see pack_inputs);
fape partial = sum_ij min(sqrt(d2+eps), 10).  Only the GLOBAL sum per
core matters (both units share b), so jobs sum in any grouping.

Per core (2 (l,b) units):
  PE : 128 rank-49 bf16 matmuls [128,512] into a 6-bank PSUM ring
       (two [128,3,512] tiles), row-group alternation per i-chunk;
       stacked bf16 feature transposes into a reserved bank; ones-colsum
       of a fraction of jobs into the 8th bank (column-group alternation).
  ACT: sqrt(d2 + 0.25) f32->bf16, [128,3,512] jobs.  The 0.25 bias
       absorbs bf16 rounding of near-zero d2 so sqrt never sees negative
       inputs (error ~0.1%, validated).
  DVE: feature products (f32, 2x), bf16 unstack copies (2x), then per
       job either fused min+sum (CACHE_REDUCE) or min-only (4x) for the
       PE-colsum route.
  GPSIMD/ACT: torsion loss (identical to baseline).
"""
import os
import sys
import numpy as np

sys.path.insert(0, "/opt/trn_rl_repo")

import concourse.bacc as bacc
import concourse.tile as tile
import concourse.mybir as mybir
from concourse.bass_utils import run_bass_kernel_spmd

f32 = mybir.dt.float32
bf16 = mybir.dt.bfloat16
ACT = mybir.ActivationFunctionType
ALU = mybir.AluOpType
AX = mybir.AxisListType

L, B, N = 8, 2, 2048
NC = 16          # i-chunks of 128
P = 128
K = 49
D_CLAMP = 10.0
SQRT_BIAS = 0.25
Z_SCALE = 10.0
TORSION_EPS = 1e-8

JOB_WIDTHS = [2] * 32                  # 64 matmuls per unit
assert sum(JOB_WIDTHS) == 64

_cache = {}


def build_program(dve_frac=0.62):
    # route: True -> DVE fused min+sum; False -> DVE min + PE colsum.
    # Colsum jobs go at the END of each unit: early in the unit the DVE
    # still drains the feature-product chain, and a colsum matmul waiting
    # on DVE's min pass would block the in-order PE queue.
    # greedy split (the best-measured interleave), same for both units
    dve_route = []
    dcols, tot = 0.0, 0.0
    for w in JOB_WIDTHS:
        dve_route.append(dcols <= dve_frac * tot + 1e-9)
        if dve_route[-1]:
            dcols += w
        tot += w
    dve_route[-1] = True
    routes = [dve_route, dve_route]

    nc = bacc.Bacc("TRN2", target_bir_lowering=False, debug=False)

    def register_const_ap(value, dtype=f32):
        t = nc.alloc_sbuf_tensor(f"const-{dtype.name}-{value}", [128, 1], dtype)
        nc.gpsimd.memset(t.ap(), value)
        nc.const_aps.aps[(dtype, value)] = t.ap()

    register_const_ap(SQRT_BIAS)
    register_const_ap(TORSION_EPS)
    nc.all_engine_barrier()

    # ---- DRAM I/O (per core)
    mrows_d = nc.dram_tensor("mrows", [P, 2, NC, 7, 3], f32, kind="ExternalInput")
    zv_d = nc.dram_tensor("zv", [P, 2, NC, 7], f32, kind="ExternalInput")
    tor_d = nc.dram_tensor("tor", [P, 2, NC, 7, 2], f32, kind="ExternalInput")
    tort_d = nc.dram_tensor("tort", [P, NC, 7, 2], f32, kind="ExternalInput")
    tora_d = nc.dram_tensor("tora", [P, NC, 7, 2], f32, kind="ExternalInput")
    ident_d = nc.dram_tensor("ident", [P, P], bf16, kind="ExternalInput")
    out_d = nc.dram_tensor("out", [1, 8], f32, kind="ExternalOutput")

    with tile.TileContext(nc) as tc:
        import contextlib
        with contextlib.ExitStack() as ctx:
            persist = ctx.enter_context(tc.tile_pool(name="persist", bufs=1))
            feat = ctx.enter_context(tc.tile_pool(name="feat", bufs=1))
            sp = ctx.enter_context(tc.tile_pool(name="sp", bufs=12))
            msp = ctx.enter_context(tc.tile_pool(name="msp", bufs=10))
            torp = ctx.enter_context(tc.tile_pool(name="torp", bufs=2))
            psA = ctx.enter_context(tc.tile_pool(name="psA", bufs=3, space="PSUM"))
            psC = ctx.enter_context(tc.tile_pool(name="psC", bufs=1, space="PSUM"))
            psD = ctx.enter_context(tc.tile_pool(name="psD", bufs=1, space="PSUM"))

            # ---- persistent inputs (spread across DMA queues)
            MT = persist.tile([P, 2, NC, 7, 3], f32, tag="mt")
            nc.sync.dma_start(MT[:, :, :, 0:6, :], mrows_d.ap()[:, :, :, 0:6, :])
            ZV = persist.tile([P, 2, NC, 7], f32, tag="zvt")
            nc.scalar.dma_start(ZV[:], zv_d.ap())
            IDN = persist.tile([P, P], bf16, tag="idn")
            nc.scalar.dma_start(IDN[:], ident_d.ap())
            TOR = persist.tile([P, 2, NC, 7, 2], f32, tag="tor")
            nc.gpsimd.dma_start(TOR[:], tor_d.ap())
            TORT = persist.tile([P, NC, 7, 2], f32, tag="tort")
            nc.gpsimd.dma_start(TORT[:], tort_d.ap())
            TORA = persist.tile([P, NC, 7, 2], f32, tag="tora")
            nc.scalar.dma_start(TORA[:], tora_d.ap())

            ACC = persist.tile([P, 2 * 40], f32, tag="acc")
            nc.vector.memset(ACC[:], 0.0)
            FIN = persist.tile([P, 8], f32, tag="fin")
            nc.vector.memset(FIN[:], 0.0)
            ONES = persist.tile([P, 1], f32, tag="ones")
            nc.vector.memset(ONES[:], 1.0)
            OBF = persist.tile([P, 32], bf16, tag="obf")
            nc.vector.memset(OBF[:], 1.0)
            WRM = persist.tile([P, 1], f32, tag="wrm")
            nc.vector.memset(WRM[:], 1.0)
            nc.scalar.activation(WRM[:], WRM[:], ACT.Sqrt, bias=SQRT_BIAS,
                                 scale=1.0)

            # PSUM: 3-deep ring of 2-bank tiles, CS 1 bank, TT scratch
            CS = psC.tile([P, 512], f32, tag="cs")

            # ---- feature tensors
            FB = [[feat.tile([P, NC, 64], bf16, tag=f"fb{u}{t}",
                             name=f"FB{u}{t}")
                   for t in range(2)] for u in range(2)]
            for u in range(2):
                nc.vector.memset(FB[u][0][:], 0.0)
                nc.vector.memset(FB[u][1][:], 0.0)
            FT = [[feat.tile([P, N], bf16, tag=f"ft{u}{t}",
                             name=f"FT{u}{t}")
                   for t in range(2)] for u in range(2)]
            FT2 = [[feat.tile([64 + K, N], bf16, tag=f"ft2{u}{t}",
                              name=f"FT2{u}{t}")
                    for t in range(2)] for u in range(2)]

            def feature_ops(u):
                """Closures emitting unit u's features + transposes.
                Products in bf16 with the chunk axis LAST so both
                outer-product operands keep a packed last dim -> DVE 4x
                mode (~0.3us/op instead of ~1.1us)."""
                eng = nc.vector
                mt_u = MT[:, u]      # [P, NC, 7, 3]
                zv_u = ZV[:, u]      # [P, NC, 7]
                Ff = feat.tile([P, NC, 7, 7], f32, tag=f"ff{u}",
                               name=f"Ff{u}")
                Fr = feat.tile([P, NC, 7, 7], f32, tag=f"fr{u}",
                               name=f"Fr{u}")
                Zf = feat.tile([P, NC, 7, 7], f32, tag=f"zf{u}",
                               name=f"Zf{u}")
                eng = nc.vector
                ops = []

                def c_row():
                    CP = feat.tile([P, NC, 3, 6], f32, tag=f"cp{u}",
                                   name=f"CP{u}")
                    nc.gpsimd.tensor_tensor(
                        CP[:],
                        mt_u[:, :, 0:6, :].rearrange("p c m r -> p c r m"),
                        zv_u[:, :, 0:6].unsqueeze(2).broadcast_to(
                            [P, NC, 3, 6]),
                        ALU.mult,
                    )
                    nc.vector.tensor_reduce(
                        mt_u[:, :, 6, :], CP[:], AX.X, ALU.add, negate=True
                    )
                ops.append(c_row)

                def zprod():
                    eng.tensor_tensor(
                        Zf[:],
                        zv_u.unsqueeze(3).broadcast_to([P, NC, 7, 7]),
                        zv_u.unsqueeze(2).broadcast_to([P, NC, 7, 7]),
                        ALU.mult,
                    )
                ops.append(zprod)

                def fprod(r):
                    dst = Ff if r == 0 else Fr
                    eng.tensor_tensor(
                        dst[:],
                        mt_u[:, :, :, r].unsqueeze(3).broadcast_to(
                            [P, NC, 7, 7]),
                        mt_u[:, :, :, r].unsqueeze(2).broadcast_to(
                            [P, NC, 7, 7]),
                        ALU.mult,
                    )
                    if r > 0:
                        eng.tensor_tensor(Ff[:], Ff[:], Fr[:], ALU.add)
                for r in range(3):
                    ops.append(lambda r=r: fprod(r))

                def castZ():
                    nc.vector.tensor_copy(
                        FB[u][1][:, :, 0:K],
                        Zf[:].rearrange("p c a b -> p c (a b)"))
                ops.append(castZ)

                def castF():
                    nc.vector.tensor_copy(
                        FB[u][0][:, :, 0:K],
                        Ff[:].rearrange("p c a b -> p c (a b)"))
                ops.append(castF)

                def tfill(t, f):
                    TT = psD.tile([P, 4, P], bf16, tag="tt", name="TT")
                    for s in range(4):   # chunks (8f+2s, 8f+2s+1)
                        c0 = 8 * f + 2 * s
                        nc.tensor.transpose(
                            TT[:, s, :],
                            FB[u][t][:, c0:c0 + 2, :].rearrange(
                                "p c k -> p (c k)"),
                            IDN[:])
                    dst = FT[u][t][0:K, f * 1024:(f + 1) * 1024]
                    dv = dst.rearrange("r (a w p) -> r a w p", w=2, p=P)
                    nc.vector.tensor_copy(dv[:, :, 0, :], TT[0:K, :, :])
                    nc.vector.tensor_copy(dv[:, :, 1, :],
                                          TT[64:64 + K, :, :])
                # Z first (rhs of every matmul), then F
                for t in (1, 0):
                    for f in range(2):
                        ops.append(lambda t=t, f=f: tfill(t, f))
                    ops.append(lambda t=t: nc.sync.dma_start(
                        FT2[u][t][64:64 + K, :], FT[u][t][0:K, :]))
                return ops

            u0_ops = feature_ops(0)
            u1_ops_all = feature_ops(1)
            for op in u0_ops:          # unit-0: everything upfront
                op()
            for op in u1_ops_all[0:7]:  # unit-1 products + casts upfront
                op()
            u1_late = u1_ops_all[7:]    # transposes + dups woven

            # ---- main loop
            state = {"ncs": 0, "nacc": 0}
            total_cs_mms = sum(
                0 if r else w
                for route in routes for r, w in zip(route, JOB_WIDTHS))

            CS_LAG = 8

            def emit_unit(u, weave_ops):
                dve_route = routes[u]
                pending = []   # (MS tile, width) awaiting colsum emission

                def emit_colsum(MSp, wp):
                    for k in range(wp):
                        cg = 64 * (state["ncs"] % 2)
                        nc.tensor.matmul(
                            CS[cg:cg + 32, :],
                            lhsT=OBF[:],
                            rhs=MSp[:, k, :],
                            start=(state["ncs"] < 2),
                            stop=(state["ncs"] >= total_cs_mms - 2),
                            tile_position=(0, cg),
                            skip_group_check=True,
                        )
                        state["ncs"] += 1

                njob = 0
                mm = 0
                for j, w in enumerate(JOB_WIDTHS):
                    tile_t = psA.tile([P, 2, 512], f32, tag="a")
                    for k in range(w):
                        m = mm + k
                        c = m // 4
                        n = m % 4
                        rg = 64 * (m % 2)
                        lhs = (FT[u][0][0:K] if rg == 0
                               else FT2[u][0][64:64 + K])
                        rhs = (FT[u][1][0:K] if rg == 0
                               else FT2[u][1][64:64 + K])
                        nc.tensor.matmul(
                            tile_t[:, k, :],
                            lhsT=lhs[:, c * P:(c + 1) * P],
                            rhs=rhs[:, n * 512:(n + 1) * 512],
                            start=True, stop=True,
                            tile_position=(rg, 0),
                        )
                    mm += w
                    S = sp.tile([P, 2, 512], bf16, tag="s")
                    nc.scalar.activation(
                        S[:, 0:w, :], tile_t[:, 0:w, :], ACT.Sqrt,
                        bias=SQRT_BIAS, scale=1.0)
                    MS = msp.tile([P, 2, 512], bf16, tag="ms")
                    if dve_route[njob]:
                        nc.vector.tensor_scalar(
                            MS[:, 0:w, :], S[:, 0:w, :], D_CLAMP, None,
                            ALU.min, ALU.add,
                            accum_out=ACC[:, u * 40 + state["nacc"]:
                                          u * 40 + state["nacc"] + 1])
                        state["nacc"] += 1
                    else:
                        nc.vector.tensor_scalar(
                            MS[:, 0:w, :], S[:, 0:w, :], 0.0, D_CLAMP,
                            ALU.max, ALU.min)
                        pending.append((MS, w, njob))
                    while pending and pending[0][2] <= njob - CS_LAG:
                        MSp, wp, _ = pending.pop(0)
                        emit_colsum(MSp, wp)
                    njob += 1
                    if weave_ops and njob >= 15 and njob % 2 == 1:
                        weave_ops.pop(0)()
                for MSp, wp, _ in pending:
                    emit_colsum(MSp, wp)
                state["nacc"] = 0

            emit_unit(0, u1_late)
            while u1_late:
                u1_late.pop(0)()
            emit_unit(1, [])

            # ---- colsum wrap-up (mixed units is fine: same b, same scale)
            if total_cs_mms > 0:
                CSUM = persist.tile([P, 2], f32, tag="csum")
                nc.vector.tensor_reduce(CSUM[0:32, 0:1], CS[0:32, :], AX.X,
                                        ALU.add)
                if total_cs_mms > 1:
                    nc.vector.tensor_reduce(CSUM[0:32, 1:2], CS[64:96, :],
                                            AX.X, ALU.add)
                else:
                    nc.vector.memset(CSUM[0:32, 1:2], 0.0)
                nc.vector.tensor_tensor(FIN[0:32, 6:7], CSUM[0:32, 0:1],
                                        CSUM[0:32, 1:2], ALU.add)

            # ---- fape partials from ACC (DVE-route accum columns)
            for u in range(2):
                FSC = torp.tile([P, 1], f32, tag=f"fsc{u}", name=f"FSC{u}")
                nc.vector.tensor_reduce(
                    FSC[:], ACC[:, u * 40:(u + 1) * 40], AX.X, ALU.add)
                nc.vector.tensor_copy(FIN[:, u:u + 1], FSC[:])

            # ---- torsion loss per unit
            for u in range(2):
                tor_u = TOR[:, u]  # [P, NC, 7, 2]
                SQ = torp.tile([P, NC, 7, 2], f32, tag="sq")
                nc.gpsimd.tensor_tensor(SQ[:], tor_u[:], tor_u[:], ALU.mult)
                N2 = torp.tile([P, NC, 7], f32, tag="n2")
                nc.gpsimd.tensor_tensor(
                    N2[:], SQ[:, :, :, 0], SQ[:, :, :, 1], ALU.add)
                NRM = torp.tile([P, NC, 7], f32, tag="nrm")
                nc.scalar.activation(NRM[:], N2[:], ACT.Sqrt,
                                     bias=TORSION_EPS, scale=1.0)
                REC = torp.tile([P, NC, 7], f32, tag="rec")
                nc.vector.reciprocal(REC[:], NRM[:])
                PN = torp.tile([P, NC, 7, 2], f32, tag="pn")
                nc.gpsimd.tensor_tensor(
                    PN[:], tor_u[:],
                    REC[:].unsqueeze(3).broadcast_to([P, NC, 7, 2]),
                    ALU.mult,
                )
                DV = []
                for name, TTRUE in (("t", TORT), ("a", TORA)):
                    DF = torp.tile([P, NC, 7, 2], f32, tag=f"df{name}")
                    nc.gpsimd.tensor_tensor(DF[:], TTRUE[:], PN[:],
                                            ALU.subtract)
                    DS = torp.tile([P, NC, 7, 2], f32, tag=f"ds{name}")
                    nc.gpsimd.tensor_tensor(DS[:], DF[:], DF[:], ALU.mult)
                    D2T = torp.tile([P, NC, 7], f32, tag=f"d2t{name}")
                    nc.gpsimd.tensor_tensor(
                        D2T[:], DS[:, :, :, 0], DS[:, :, :, 1], ALU.add)
                    DVt = torp.tile([P, NC, 7], f32, tag=f"dv{name}")
                    nc.scalar.activation(DVt[:], D2T[:], ACT.Sqrt,
                                         bias=TORSION_EPS, scale=1.0)
                    DV.append(DVt)
                MN = torp.tile([P, NC, 7], f32, tag="mn")
                nc.vector.tensor_tensor(MN[:], DV[0][:], DV[1][:], ALU.min)
                nc.vector.tensor_reduce(FIN[:, 2 + u:3 + u], MN[:], AX.XY,
                                        ALU.add)
                AN = torp.tile([P, NC, 7], f32, tag="an")
                nc.vector.tensor_scalar(AN[:], NRM[:], 1.0, None,
                                        ALU.subtract)
                nc.vector.tensor_reduce(
                    FIN[:, 4 + u:5 + u], AN[:], AX.XY, ALU.add,
                    apply_absolute_value=True,
                )

            # ---- cross-partition reduce via ones-matmul
            fin_ps = psA.tile([P, 2, 512], f32, tag="a")
            nc.tensor.matmul(
                fin_ps[0:1, 0, 0:8],
                lhsT=ONES[:],
                rhs=FIN[:],
                start=True, stop=True,
            )
            OUT = persist.tile([1, 8], f32, tag="out")
            nc.scalar.copy(OUT[:], fin_ps[0:1, 0, 0:8])
            nc.sync.dma_start(out_d.ap(), OUT[:])

    nc.compile()
    return nc


def pack_inputs(traj_rotations, traj_translations, traj_torsions,
                true_rotations, true_translations,
                true_torsion_angles, true_torsion_angles_alt):
    """Build the 8 per-core input maps (host-side shard + layout)."""

    def chunked(x):
        # [N, ...] -> [P, NC, ...]  with i = c*128 + p
        return np.ascontiguousarray(
            x.reshape(NC, P, *x.shape[1:]).transpose(1, 0, *range(2, x.ndim + 1))
        )

    ident = np.eye(P, dtype=np.float32).astype(mybir.dt.np(bf16))
    in_maps = []
    for k in range(8):
        b = k // 4
        ls = [(2 * k) % 8, (2 * k) % 8 + 1]
        mrows = np.zeros((P, 2, NC, 7, 3), np.float32)
        zv = np.zeros((P, 2, NC, 7), np.float32)
        tor = np.zeros((P, 2, NC, 7, 2), np.float32)
        for u, l in enumerate(ls):
            mrows[:, u, :, 0:3, :] = chunked(traj_rotations[l, b])
            mrows[:, u, :, 3:6, :] = -chunked(true_rotations[b])
            zv[:, u, :, 0:3] = chunked(traj_translations[l, b])
            zv[:, u, :, 3:6] = chunked(true_translations[b])
            zv[:, u, :, 6] = 1.0
            tor[:, u] = chunked(traj_torsions[l, b])
        in_maps.append({
            "mrows": mrows,
            "zv": zv,
            "tor": tor,
            "tort": chunked(true_torsion_angles[b]),
            "tora": chunked(true_torsion_angles_alt[b]),
            "ident": ident,
        })
    return in_maps


def combine_outputs(results):
    """results: list of 8 dicts with 'out' [1,8] -> full output [B] f32."""
    total = np.zeros(B, np.float64)
    for k in range(8):
        b = k // 4
        o = results[k]["out"][0].astype(np.float64)
        for u in range(2):
            fape = o[u] / (N * N) / Z_SCALE
            tor = o[2 + u] / (7 * N) + 0.02 * o[4 + u] / (7 * N)
            total[b] += fape + tor
        # colsum partial (both units; 32 identical ones-matmul rows)
        total[b] += o[6] / 32.0 / (N * N) / Z_SCALE
    return (total / L).astype(np.float32)


def _install_ntff_shim():
    """The image's antenv lacks axon_hooks; synthesize it so trace=True can
    drive NTFF profiling via the ctypes hook in trn_agent_boot."""
    import types
    if "antenv.axon_hooks" in sys.modules:
        return
    try:
        from trn_agent_boot.trn_boot import _ntff_profile_via_ctypes
        hook = _ntff_profile_via_ctypes("/opt/axon/libaxon_pjrt.so")
    except Exception:
        hook = None
    mod = types.ModuleType("antenv.axon_hooks")
    mod._hook = hook
    mod.get_axon_ntff_profile_hook = lambda: mod._hook
    mod.set_axon_ntff_profile_hook = lambda h: setattr(mod, "_hook", h)
    sys.modules["antenv.axon_hooks"] = mod


def kernel(**inputs):
    if "nc" not in _cache:
        _cache["nc"] = build_program(
            float(os.environ.get("KERNEL_DVE_FRAC", "0.62")))
    nc = _cache["nc"]
    in_maps = pack_inputs(**{k: np.asarray(v) for k, v in inputs.items()})
    trace = bool(int(os.environ.get("KERNEL_TRACE", "0")))
    if trace:
        _install_ntff_shim()
    res = run_bass_kernel_spmd(
        nc, in_maps, list(range(8)),
        trace=trace,
    )
    _cache["last_results"] = res
    return combine_outputs(res.results)


# revision 5
# speedup vs baseline: 1.0366x; 1.0366x over previous
"""Trainium2 Bass kernel v2 for nn_AuxiliaryLoss (FAPE + torsion loss).

Math: d2(i,j) = <F_i, Z_j> rank-49 quadratic form (see pack_inputs);
fape partial = sum_ij min(sqrt(d2+eps), 10).  Only the GLOBAL sum per
core matters (both units share b), so jobs sum in any grouping.

Per core (2 (l,b) units):
  PE : 128 rank-49 bf16 matmuls [128,512] into a 6-bank PSUM ring
       (two [128,3,512] tiles), row-group alternation per i-chunk;
       stacked bf16 feature transposes into a reserved bank; ones-colsum
       of a fraction of jobs into the 8th bank (column-group alternation).
  ACT: sqrt(d2 + 0.25) f32->bf16, [128,3,512] jobs.  The 0.25 bias
       absorbs bf16 rounding of near-zero d2 so sqrt never sees negative
       inputs (error ~0.1%, validated).
  DVE: feature products (f32, 2x), bf16 unstack copies (2x), then per
       job either fused min+sum (CACHE_REDUCE) or min-only (4x) for the
       PE-colsum route.
  GPSIMD/ACT: torsion loss (identical to baseline).
"""
import os
import sys
import numpy as np

sys.path.insert(0, "/opt/trn_rl_repo")

import concourse.bacc as bacc
import concourse.tile as tile
import concourse.mybir as mybir
from concourse.bass_utils import run_bass_kernel_spmd

f32 = mybir.dt.float32
bf16 = mybir.dt.bfloat16
ACT = mybir.ActivationFunctionType
ALU = mybir.AluOpType
AX = mybir.AxisListType

L, B, N = 8, 2, 2048
NC = 16          # i-chunks of 128
P = 128
K = 49
D_CLAMP = 10.0
SQRT_BIAS = 0.25
Z_SCALE = 10.0
TORSION_EPS = 1e-8

JOB_WIDTHS = [2] * 32                  # 64 matmuls per unit
assert sum(JOB_WIDTHS) == 64

_cache = {}


def build_program(dve_frac=0.62):
    # route: True -> DVE fused min+sum; False -> DVE min + PE colsum.
    # Colsum jobs go at the END of each unit: early in the unit the DVE
    # still drains the feature-product chain, and a colsum matmul waiting
    # on DVE's min pass would block the in-order PE queue.
    # greedy split (the best-measured interleave), same for both units
    dve_route = []
    dcols, tot = 0.0, 0.0
    for w in JOB_WIDTHS:
        dve_route.append(dcols <= dve_frac * tot + 1e-9)
        if dve_route[-1]:
            dcols += w
        tot += w
    dve_route[-1] = True
    routes = [dve_route, dve_route]

    nc = bacc.Bacc("TRN2", target_bir_lowering=False, debug=False)

    def register_const_ap(value, dtype=f32):
        t = nc.alloc_sbuf_tensor(f"const-{dtype.name}-{value}", [128, 1], dtype)
        nc.gpsimd.memset(t.ap(), value)
        nc.const_aps.aps[(dtype, value)] = t.ap()

    register_const_ap(SQRT_BIAS)
    register_const_ap(TORSION_EPS)
    nc.all_engine_barrier()

    # ---- DRAM I/O (per core)
    mrows_d = nc.dram_tensor("mrows", [P, 2, NC, 7, 3], f32, kind="ExternalInput")
    zv_d = nc.dram_tensor("zv", [P, 2, NC, 7], f32, kind="ExternalInput")
    tor_d = nc.dram_tensor("tor", [P, 2, NC, 7, 2], f32, kind="ExternalInput")
    tort_d = nc.dram_tensor("tort", [P, NC, 7, 2], f32, kind="ExternalInput")
    tora_d = nc.dram_tensor("tora", [P, NC, 7, 2], f32, kind="ExternalInput")
    ident_d = nc.dram_tensor("ident", [P, P], bf16, kind="ExternalInput")
    out_d = nc.dram_tensor("out", [1, 8], f32, kind="ExternalOutput")

    with tile.TileContext(nc) as tc:
        import contextlib
        with contextlib.ExitStack() as ctx:
            persist = ctx.enter_context(tc.tile_pool(name="persist", bufs=1))
            feat = ctx.enter_context(tc.tile_pool(name="feat", bufs=1))
            sp = ctx.enter_context(tc.tile_pool(name="sp", bufs=12))
            msp = ctx.enter_context(tc.tile_pool(name="msp", bufs=12))
            torp = ctx.enter_context(tc.tile_pool(name="torp", bufs=2))
            psA = ctx.enter_context(tc.tile_pool(name="psA", bufs=3, space="PSUM"))
            psC = ctx.enter_context(tc.tile_pool(name="psC", bufs=1, space="PSUM"))
            psD = ctx.enter_context(tc.tile_pool(name="psD", bufs=1, space="PSUM"))

            # ---- persistent inputs (spread across DMA queues)
            MT = persist.tile([P, 2, NC, 7, 3], f32, tag="mt")
            nc.sync.dma_start(MT[:, :, :, 0:6, :], mrows_d.ap()[:, :, :, 0:6, :])
            ZV = persist.tile([P, 2, NC, 7], f32, tag="zvt")
            nc.scalar.dma_start(ZV[:], zv_d.ap())
            IDN = persist.tile([P, P], bf16, tag="idn")
            nc.scalar.dma_start(IDN[:], ident_d.ap())
            TOR = persist.tile([P, 2, NC, 7, 2], f32, tag="tor")
            nc.gpsimd.dma_start(TOR[:], tor_d.ap())
            TORT = persist.tile([P, NC, 7, 2], f32, tag="tort")
            nc.gpsimd.dma_start(TORT[:], tort_d.ap())
            TORA = persist.tile([P, NC, 7, 2], f32, tag="tora")
            nc.scalar.dma_start(TORA[:], tora_d.ap())

            ACC = persist.tile([P, 2 * 40], f32, tag="acc")
            nc.vector.memset(ACC[:], 0.0)
            FIN = persist.tile([P, 8], f32, tag="fin")
            nc.vector.memset(FIN[:], 0.0)
            ONES = persist.tile([P, 1], f32, tag="ones")
            nc.vector.memset(ONES[:], 1.0)
            OBF = persist.tile([P, 32], bf16, tag="obf")
            nc.vector.memset(OBF[:], 1.0)
            WRM = persist.tile([P, 1], f32, tag="wrm")
            nc.vector.memset(WRM[:], 1.0)
            nc.scalar.activation(WRM[:], WRM[:], ACT.Sqrt, bias=SQRT_BIAS,
                                 scale=1.0)

            # PSUM: 3-deep ring of 2-bank tiles, CS 1 bank, TT scratch
            CS = psC.tile([P, 512], f32, tag="cs")

            # ---- feature tensors
            FB = [[feat.tile([P, NC, 64], bf16, tag=f"fb{u}{t}",
                             name=f"FB{u}{t}")
                   for t in range(2)] for u in range(2)]
            for u in range(2):
                nc.vector.memset(FB[u][0][:], 0.0)
                nc.vector.memset(FB[u][1][:], 0.0)
            FT = [[feat.tile([P, N], bf16, tag=f"ft{u}{t}",
                             name=f"FT{u}{t}")
                   for t in range(2)] for u in range(2)]
            FT2 = [[feat.tile([64 + K, N], bf16, tag=f"ft2{u}{t}",
                              name=f"FT2{u}{t}")
                    for t in range(2)] for u in range(2)]

            def feature_ops(u):
                """Closures emitting unit u's features + transposes.
                Products in bf16 with the chunk axis LAST so both
                outer-product operands keep a packed last dim -> DVE 4x
                mode (~0.3us/op instead of ~1.1us)."""
                eng = nc.vector
                mt_u = MT[:, u]      # [P, NC, 7, 3]
                zv_u = ZV[:, u]      # [P, NC, 7]
                Ff = feat.tile([P, NC, 7, 7], f32, tag=f"ff{u}",
                               name=f"Ff{u}")
                Fr = feat.tile([P, NC, 7, 7], f32, tag=f"fr{u}",
                               name=f"Fr{u}")
                Zf = feat.tile([P, NC, 7, 7], f32, tag=f"zf{u}",
                               name=f"Zf{u}")
                eng = nc.vector
                ops = []

                def c_row():
                    CP = feat.tile([P, NC, 3, 6], f32, tag=f"cp{u}",
                                   name=f"CP{u}")
                    nc.gpsimd.tensor_tensor(
                        CP[:],
                        mt_u[:, :, 0:6, :].rearrange("p c m r -> p c r m"),
                        zv_u[:, :, 0:6].unsqueeze(2).broadcast_to(
                            [P, NC, 3, 6]),
                        ALU.mult,
                    )
                    nc.vector.tensor_reduce(
                        mt_u[:, :, 6, :], CP[:], AX.X, ALU.add, negate=True
                    )
                ops.append(c_row)

                def zprod():
                    eng.tensor_tensor(
                        Zf[:],
                        zv_u.unsqueeze(3).broadcast_to([P, NC, 7, 7]),
                        zv_u.unsqueeze(2).broadcast_to([P, NC, 7, 7]),
                        ALU.mult,
                    )
                ops.append(zprod)

                def fprod(r):
                    dst = Ff if r == 0 else Fr
                    eng.tensor_tensor(
                        dst[:],
                        mt_u[:, :, :, r].unsqueeze(3).broadcast_to(
                            [P, NC, 7, 7]),
                        mt_u[:, :, :, r].unsqueeze(2).broadcast_to(
                            [P, NC, 7, 7]),
                        ALU.mult,
                    )
                    if r > 0:
                        eng.tensor_tensor(Ff[:], Ff[:], Fr[:], ALU.add)
                for r in range(3):
                    ops.append(lambda r=r: fprod(r))

                def castZ():
                    nc.vector.tensor_copy(
                        FB[u][1][:, :, 0:K],
                        Zf[:].rearrange("p c a b -> p c (a b)"))
                ops.append(castZ)

                def castF():
                    nc.vector.tensor_copy(
                        FB[u][0][:, :, 0:K],
                        Ff[:].rearrange("p c a b -> p c (a b)"))
                ops.append(castF)

                def tfill(t, f):
                    TT = psD.tile([P, 4, P], bf16, tag="tt", name="TT")
                    for s in range(4):   # chunks (8f+2s, 8f+2s+1)
                        c0 = 8 * f + 2 * s
                        nc.tensor.transpose(
                            TT[:, s, :],
                            FB[u][t][:, c0:c0 + 2, :].rearrange(
                                "p c k -> p (c k)"),
                            IDN[:])
                    dst = FT[u][t][0:K, f * 1024:(f + 1) * 1024]
                    dv = dst.rearrange("r (a w p) -> r a w p", w=2, p=P)
                    nc.vector.tensor_copy(dv[:, :, 0, :], TT[0:K, :, :])
                    nc.vector.tensor_copy(dv[:, :, 1, :],
                                          TT[64:64 + K, :, :])
                # Z first (rhs of every matmul), then F
                for t in (1, 0):
                    for f in range(2):
                        ops.append(lambda t=t, f=f: tfill(t, f))
                    ops.append(lambda t=t: nc.sync.dma_start(
                        FT2[u][t][64:64 + K, :], FT[u][t][0:K, :]))
                return ops

            u0_ops = feature_ops(0)
            u1_ops_all = feature_ops(1)
            for op in u0_ops:          # unit-0: everything upfront
                op()
            for op in u1_ops_all[0:7]:  # unit-1 products + casts upfront
                op()
            u1_late = u1_ops_all[7:]    # transposes + dups woven

            # ---- main loop
            state = {"ncs": 0, "nacc": 0}
            total_cs_mms = sum(
                0 if r else w
                for route in routes for r, w in zip(route, JOB_WIDTHS))

            CS_LAG = 16

            def emit_unit(u, weave_ops):
                dve_route = routes[u]
                pending = []   # (MS tile, width) awaiting colsum emission

                def emit_colsum(MSp, wp):
                    for k in range(wp):
                        cg = 64 * (state["ncs"] % 2)
                        nc.tensor.matmul(
                            CS[cg:cg + 32, :],
                            lhsT=OBF[:],
                            rhs=MSp[:, k, :],
                            start=(state["ncs"] < 2),
                            stop=(state["ncs"] >= total_cs_mms - 2),
                            tile_position=(0, cg),
                            skip_group_check=True,
                        )
                        state["ncs"] += 1

                njob = 0
                mm = 0
                for j, w in enumerate(JOB_WIDTHS):
                    tile_t = psA.tile([P, 2, 512], f32, tag="a")
                    for k in range(w):
                        m = mm + k
                        c = m // 4
                        n = m % 4
                        rg = 64 * (m % 2)
                        lhs = (FT[u][0][0:K] if rg == 0
                               else FT2[u][0][64:64 + K])
                        rhs = (FT[u][1][0:K] if rg == 0
                               else FT2[u][1][64:64 + K])
                        nc.tensor.matmul(
                            tile_t[:, k, :],
                            lhsT=lhs[:, c * P:(c + 1) * P],
                            rhs=rhs[:, n * 512:(n + 1) * 512],
                            start=True, stop=True,
                            tile_position=(rg, 0),
                        )
                    mm += w
                    S = sp.tile([P, 2, 512], bf16, tag="s")
                    nc.scalar.activation(
                        S[:, 0:w, :], tile_t[:, 0:w, :], ACT.Sqrt,
                        bias=SQRT_BIAS, scale=1.0)
                    MS = msp.tile([P, 2, 512], bf16, tag="ms")
                    if dve_route[njob]:
                        nc.vector.tensor_scalar(
                            MS[:, 0:w, :], S[:, 0:w, :], D_CLAMP, None,
                            ALU.min, ALU.add,
                            accum_out=ACC[:, u * 40 + state["nacc"]:
                                          u * 40 + state["nacc"] + 1])
                        state["nacc"] += 1
                    else:
                        nc.vector.tensor_scalar(
                            MS[:, 0:w, :], S[:, 0:w, :], 0.0, D_CLAMP,
                            ALU.max, ALU.min)
                        pending.append((MS, w, njob))
                    while pending and pending[0][2] <= njob - CS_LAG:
                        MSp, wp, _ = pending.pop(0)
                        emit_colsum(MSp, wp)
                    njob += 1
                    if weave_ops and njob >= 15 and njob % 2 == 1:
                        weave_ops.pop(0)()
                for MSp, wp, _ in pending:
                    emit_colsum(MSp, wp)
                state["nacc"] = 0

            # ---- torsion loss per unit (emitted early for overlap)
            def emit_torsion(u):
                tor_u = TOR[:, u]  # [P, NC, 7, 2]
                SQ = torp.tile([P, NC, 7, 2], f32, tag="sq")
                nc.gpsimd.tensor_tensor(SQ[:], tor_u[:], tor_u[:], ALU.mult)
                N2 = torp.tile([P, NC, 7], f32, tag="n2")
                nc.gpsimd.tensor_tensor(
                    N2[:], SQ[:, :, :, 0], SQ[:, :, :, 1], ALU.add)
                NRM = torp.tile([P, NC, 7], f32, tag="nrm")
                nc.scalar.activation(NRM[:], N2[:], ACT.Sqrt,
                                     bias=TORSION_EPS, scale=1.0)
                REC = torp.tile([P, NC, 7], f32, tag="rec")
                nc.vector.reciprocal(REC[:], NRM[:])
                PN = torp.tile([P, NC, 7, 2], f32, tag="pn")
                nc.gpsimd.tensor_tensor(
                    PN[:], tor_u[:],
                    REC[:].unsqueeze(3).broadcast_to([P, NC, 7, 2]),
                    ALU.mult,
                )
                DV = []
                for name, TTRUE in (("t", TORT), ("a", TORA)):
                    DF = torp.tile([P, NC, 7, 2], f32, tag=f"df{name}")
                    nc.gpsimd.tensor_tensor(DF[:], TTRUE[:], PN[:],
                                            ALU.subtract)
                    DS = torp.tile([P, NC, 7, 2], f32, tag=f"ds{name}")
                    nc.gpsimd.tensor_tensor(DS[:], DF[:], DF[:], ALU.mult)
                    D2T = torp.tile([P, NC, 7], f32, tag=f"d2t{name}")
                    nc.gpsimd.tensor_tensor(
                        D2T[:], DS[:, :, :, 0], DS[:, :, :, 1], ALU.add)
                    DVt = torp.tile([P, NC, 7], f32, tag=f"dv{name}")
                    nc.scalar.activation(DVt[:], D2T[:], ACT.Sqrt,
                                         bias=TORSION_EPS, scale=1.0)
                    DV.append(DVt)
                MN = torp.tile([P, NC, 7], f32, tag="mn")
                nc.vector.tensor_tensor(MN[:], DV[0][:], DV[1][:], ALU.min)
                nc.vector.tensor_reduce(FIN[:, 2 + u:3 + u], MN[:], AX.XY,
                                        ALU.add)
                AN = torp.tile([P, NC, 7], f32, tag="an")
                nc.vector.tensor_scalar(AN[:], NRM[:], 1.0, None,
                                        ALU.subtract)
                nc.vector.tensor_reduce(
                    FIN[:, 4 + u:5 + u], AN[:], AX.XY, ALU.add,
                    apply_absolute_value=True,
                )


            emit_unit(0, u1_late)
            while u1_late:
                u1_late.pop(0)()
            emit_torsion(0)
            emit_unit(1, [])

            # ---- colsum wrap-up (mixed units is fine: same b, same scale)
            if total_cs_mms > 0:
                CSUM = persist.tile([P, 2], f32, tag="csum")
                nc.vector.tensor_reduce(CSUM[0:32, 0:1], CS[0:32, :], AX.X,
                                        ALU.add)
                if total_cs_mms > 1:
                    nc.vector.tensor_reduce(CSUM[0:32, 1:2], CS[64:96, :],
                                            AX.X, ALU.add)
                else:
                    nc.vector.memset(CSUM[0:32, 1:2], 0.0)
                nc.vector.tensor_tensor(FIN[0:32, 6:7], CSUM[0:32, 0:1],
                                        CSUM[0:32, 1:2], ALU.add)

            # ---- fape partials from ACC (DVE-route accum columns)
            for u in range(2):
                FSC = torp.tile([P, 1], f32, tag=f"fsc{u}", name=f"FSC{u}")
                nc.vector.tensor_reduce(
                    FSC[:], ACC[:, u * 40:(u + 1) * 40], AX.X, ALU.add)
                nc.vector.tensor_copy(FIN[:, u:u + 1], FSC[:])

            emit_torsion(1)

            # ---- cross-partition reduce via ones-matmul
            fin_ps = psA.tile([P, 2, 512], f32, tag="a")
            nc.tensor.matmul(
                fin_ps[0:1, 0, 0:8],
                lhsT=ONES[:],
                rhs=FIN[:],
                start=True, stop=True,
            )
            OUT = persist.tile([1, 8], f32, tag="out")
            nc.scalar.copy(OUT[:], fin_ps[0:1, 0, 0:8])
            nc.sync.dma_start(out_d.ap(), OUT[:])

    nc.compile()
    return nc


def pack_inputs(traj_rotations, traj_translations, traj_torsions,
                true_rotations, true_translations,
                true_torsion_angles, true_torsion_angles_alt):
    """Build the 8 per-core input maps (host-side shard + layout)."""

    def chunked(x):
        # [N, ...] -> [P, NC, ...]  with i = c*128 + p
        return np.ascontiguousarray(
            x.reshape(NC, P, *x.shape[1:]).transpose(1, 0, *range(2, x.ndim + 1))
        )

    ident = np.eye(P, dtype=np.float32).astype(mybir.dt.np(bf16))
    in_maps = []
    for k in range(8):
        b = k // 4
        ls = [(2 * k) % 8, (2 * k) % 8 + 1]
        mrows = np.zeros((P, 2, NC, 7, 3), np.float32)
        zv = np.zeros((P, 2, NC, 7), np.float32)
        tor = np.zeros((P, 2, NC, 7, 2), np.float32)
        for u, l in enumerate(ls):
            mrows[:, u, :, 0:3, :] = chunked(traj_rotations[l, b])
            mrows[:, u, :, 3:6, :] = -chunked(true_rotations[b])
            zv[:, u, :, 0:3] = chunked(traj_translations[l, b])
            zv[:, u, :, 3:6] = chunked(true_translations[b])
            zv[:, u, :, 6] = 1.0
            tor[:, u] = chunked(traj_torsions[l, b])
        in_maps.append({
            "mrows": mrows,
            "zv": zv,
            "tor": tor,
            "tort": chunked(true_torsion_angles[b]),
            "tora": chunked(true_torsion_angles_alt[b]),
            "ident": ident,
        })
    return in_maps


def combine_outputs(results):
    """results: list of 8 dicts with 'out' [1,8] -> full output [B] f32."""
    total = np.zeros(B, np.float64)
    for k in range(8):
        b = k // 4
        o = results[k]["out"][0].astype(np.float64)
        for u in range(2):
            fape = o[u] / (N * N) / Z_SCALE
            tor = o[2 + u] / (7 * N) + 0.02 * o[4 + u] / (7 * N)
            total[b] += fape + tor
        # colsum partial (both units; 32 identical ones-matmul rows)
        total[b] += o[6] / 32.0 / (N * N) / Z_SCALE
    return (total / L).astype(np.float32)


def _install_ntff_shim():
    """The image's antenv lacks axon_hooks; synthesize it so trace=True can
    drive NTFF profiling via the ctypes hook in trn_agent_boot."""
    import types
    if "antenv.axon_hooks" in sys.modules:
        return
    try:
        from trn_agent_boot.trn_boot import _ntff_profile_via_ctypes
        hook = _ntff_profile_via_ctypes("/opt/axon/libaxon_pjrt.so")
    except Exception:
        hook = None
    mod = types.ModuleType("antenv.axon_hooks")
    mod._hook = hook
    mod.get_axon_ntff_profile_hook = lambda: mod._hook
    mod.set_axon_ntff_profile_hook = lambda h: setattr(mod, "_hook", h)
    sys.modules["antenv.axon_hooks"] = mod


def kernel(**inputs):
    if "nc" not in _cache:
        _cache["nc"] = build_program(
            float(os.environ.get("KERNEL_DVE_FRAC", "0.62")))
    nc = _cache["nc"]
    in_maps = pack_inputs(**{k: np.asarray(v) for k, v in inputs.items()})
    trace = bool(int(os.environ.get("KERNEL_TRACE", "0")))
    if trace:
        _install_ntff_shim()
    res = run_bass_kernel_spmd(
        nc, in_maps, list(range(8)),
        trace=trace,
    )
    _cache["last_results"] = res
    return combine_outputs(res.results)


# revision 6
# speedup vs baseline: 1.1290x; 1.0891x over previous
"""Trainium2 Bass kernel v2 for nn_AuxiliaryLoss (FAPE + torsion loss).

Math: d2(i,j) = <F_i, Z_j> rank-49 quadratic form (see pack_inputs);
fape partial = sum_ij min(sqrt(d2+eps), 10).  Only the GLOBAL sum per
core matters (both units share b), so jobs sum in any grouping.

Per core (2 (l,b) units):
  PE : 128 rank-49 bf16 matmuls [128,512] into a 6-bank PSUM ring
       (two [128,3,512] tiles), row-group alternation per i-chunk;
       stacked bf16 feature transposes into a reserved bank; ones-colsum
       of a fraction of jobs into the 8th bank (column-group alternation).
  ACT: sqrt(d2 + 0.25) f32->bf16, [128,3,512] jobs.  The 0.25 bias
       absorbs bf16 rounding of near-zero d2 so sqrt never sees negative
       inputs (error ~0.1%, validated).
  DVE: feature products (f32, 2x), bf16 unstack copies (2x), then per
       job either fused min+sum (CACHE_REDUCE) or min-only (4x) for the
       PE-colsum route.
  GPSIMD/ACT: torsion loss (identical to baseline).
"""
import os
import sys
import numpy as np

sys.path.insert(0, "/opt/trn_rl_repo")

import concourse.bacc as bacc
import concourse.tile as tile
import concourse.mybir as mybir
from concourse.bass_utils import run_bass_kernel_spmd

f32 = mybir.dt.float32
bf16 = mybir.dt.bfloat16
ACT = mybir.ActivationFunctionType
ALU = mybir.AluOpType
AX = mybir.AxisListType

L, B, N = 8, 2, 2048
NC = 16          # i-chunks of 128
P = 128
K = 49
D_CLAMP = 10.0
SQRT_BIAS = 0.25
Z_SCALE = 10.0
TORSION_EPS = 1e-8

JOB_WIDTHS = [2] * 32                  # 64 matmuls per unit
assert sum(JOB_WIDTHS) == 64

_cache = {}


def build_program(dve_frac=0.62):
    # route: True -> DVE fused min+sum; False -> DVE min + PE colsum.
    # Colsum jobs go at the END of each unit: early in the unit the DVE
    # still drains the feature-product chain, and a colsum matmul waiting
    # on DVE's min pass would block the in-order PE queue.
    # greedy split (the best-measured interleave), same for both units
    dve_route = []
    dcols, tot = 0.0, 0.0
    for w in JOB_WIDTHS:
        dve_route.append(dcols <= dve_frac * tot + 1e-9)
        if dve_route[-1]:
            dcols += w
        tot += w
    dve_route[-1] = True
    routes = [dve_route, dve_route]

    nc = bacc.Bacc("TRN2", target_bir_lowering=False, debug=False)

    def register_const_ap(value, dtype=f32):
        t = nc.alloc_sbuf_tensor(f"const-{dtype.name}-{value}", [128, 1], dtype)
        nc.gpsimd.memset(t.ap(), value)
        nc.const_aps.aps[(dtype, value)] = t.ap()

    register_const_ap(SQRT_BIAS)
    register_const_ap(TORSION_EPS)
    nc.all_engine_barrier()

    # ---- DRAM I/O (per core); feats = host-packed transposed F/Z
    feats_d = nc.dram_tensor("feats", [2, 2, K, N], bf16, kind="ExternalInput")
    tor_d = nc.dram_tensor("tor", [P, 2, NC, 7, 2], f32, kind="ExternalInput")
    tort_d = nc.dram_tensor("tort", [P, NC, 7, 2], f32, kind="ExternalInput")
    tora_d = nc.dram_tensor("tora", [P, NC, 7, 2], f32, kind="ExternalInput")
    out_d = nc.dram_tensor("out", [1, 8], f32, kind="ExternalOutput")

    with tile.TileContext(nc) as tc:
        import contextlib
        with contextlib.ExitStack() as ctx:
            persist = ctx.enter_context(tc.tile_pool(name="persist", bufs=1))
            feat = ctx.enter_context(tc.tile_pool(name="feat", bufs=1))
            sp = ctx.enter_context(tc.tile_pool(name="sp", bufs=12))
            msp = ctx.enter_context(tc.tile_pool(name="msp", bufs=12))
            torp = ctx.enter_context(tc.tile_pool(name="torp", bufs=2))
            psA = ctx.enter_context(tc.tile_pool(name="psA", bufs=3, space="PSUM"))
            psC = ctx.enter_context(tc.tile_pool(name="psC", bufs=1, space="PSUM"))

            # ---- persistent inputs (spread across DMA queues)
            TOR = persist.tile([P, 2, NC, 7, 2], f32, tag="tor")
            nc.gpsimd.dma_start(TOR[:], tor_d.ap())
            TORT = persist.tile([P, NC, 7, 2], f32, tag="tort")
            nc.gpsimd.dma_start(TORT[:], tort_d.ap())
            TORA = persist.tile([P, NC, 7, 2], f32, tag="tora")
            nc.scalar.dma_start(TORA[:], tora_d.ap())

            ACC = persist.tile([P, 2 * 40], f32, tag="acc")
            nc.vector.memset(ACC[:], 0.0)
            FIN = persist.tile([P, 8], f32, tag="fin")
            nc.vector.memset(FIN[:], 0.0)
            ONES = persist.tile([P, 1], f32, tag="ones")
            nc.vector.memset(ONES[:], 1.0)
            OBF = persist.tile([P, 32], bf16, tag="obf")
            nc.vector.memset(OBF[:], 1.0)
            WRM = persist.tile([P, 1], f32, tag="wrm")
            nc.vector.memset(WRM[:], 1.0)
            nc.scalar.activation(WRM[:], WRM[:], ACT.Sqrt, bias=SQRT_BIAS,
                                 scale=1.0)

            # PSUM: 3-deep ring of 2-bank tiles, CS 1 bank, TT scratch
            CS = psC.tile([P, 512], f32, tag="cs")

            # ---- feature tensors: loaded pre-transposed from host
            FT = [[feat.tile([P, N], bf16, tag=f"ft{u}{t}",
                             name=f"FT{u}{t}")
                   for t in range(2)] for u in range(2)]
            FT2 = [[feat.tile([64 + K, N], bf16, tag=f"ft2{u}{t}",
                              name=f"FT2{u}{t}")
                    for t in range(2)] for u in range(2)]
            qs = [nc.sync, nc.scalar, nc.gpsimd, nc.sync]
            for u in range(2):
                for t in range(2):
                    qs[2 * u + t].dma_start(FT[u][t][0:K, :],
                                            feats_d.ap()[u, t])
                    qs[2 * t + u].dma_start(FT2[u][t][64:64 + K, :],
                                            feats_d.ap()[u, t])

            # ---- main loop
            state = {"ncs": 0, "nacc": 0}
            total_cs_mms = sum(
                0 if r else w
                for route in routes for r, w in zip(route, JOB_WIDTHS))

            CS_LAG = 16

            def emit_unit(u, weave_ops):
                dve_route = routes[u]
                pending = []   # (MS tile, width) awaiting colsum emission

                def emit_colsum(MSp, wp):
                    for k in range(wp):
                        cg = 64 * (state["ncs"] % 2)
                        nc.tensor.matmul(
                            CS[cg:cg + 32, :],
                            lhsT=OBF[:],
                            rhs=MSp[:, k, :],
                            start=(state["ncs"] < 2),
                            stop=(state["ncs"] >= total_cs_mms - 2),
                            tile_position=(0, cg),
                            skip_group_check=True,
                        )
                        state["ncs"] += 1

                njob = 0
                mm = 0
                for j, w in enumerate(JOB_WIDTHS):
                    tile_t = psA.tile([P, 2, 512], f32, tag="a")
                    for k in range(w):
                        m = mm + k
                        c = m // 4
                        n = m % 4
                        rg = 64 * (m % 2)
                        lhs = (FT[u][0][0:K] if rg == 0
                               else FT2[u][0][64:64 + K])
                        rhs = (FT[u][1][0:K] if rg == 0
                               else FT2[u][1][64:64 + K])
                        nc.tensor.matmul(
                            tile_t[:, k, :],
                            lhsT=lhs[:, c * P:(c + 1) * P],
                            rhs=rhs[:, n * 512:(n + 1) * 512],
                            start=True, stop=True,
                            tile_position=(rg, 0),
                        )
                    mm += w
                    S = sp.tile([P, 2, 512], bf16, tag="s")
                    nc.scalar.activation(
                        S[:, 0:w, :], tile_t[:, 0:w, :], ACT.Sqrt,
                        bias=SQRT_BIAS, scale=1.0)
                    MS = msp.tile([P, 2, 512], bf16, tag="ms")
                    if dve_route[njob]:
                        nc.vector.tensor_scalar(
                            MS[:, 0:w, :], S[:, 0:w, :], D_CLAMP, None,
                            ALU.min, ALU.add,
                            accum_out=ACC[:, u * 40 + state["nacc"]:
                                          u * 40 + state["nacc"] + 1])
                        state["nacc"] += 1
                    else:
                        nc.vector.tensor_scalar(
                            MS[:, 0:w, :], S[:, 0:w, :], 0.0, D_CLAMP,
                            ALU.max, ALU.min)
                        pending.append((MS, w, njob))
                    while pending and pending[0][2] <= njob - CS_LAG:
                        MSp, wp, _ = pending.pop(0)
                        emit_colsum(MSp, wp)
                    njob += 1
                    if weave_ops and njob >= 15 and njob % 2 == 1:
                        weave_ops.pop(0)()
                for MSp, wp, _ in pending:
                    emit_colsum(MSp, wp)
                state["nacc"] = 0

            # ---- torsion loss per unit (emitted early for overlap)
            def emit_torsion(u):
                tor_u = TOR[:, u]  # [P, NC, 7, 2]
                SQ = torp.tile([P, NC, 7, 2], f32, tag="sq")
                nc.gpsimd.tensor_tensor(SQ[:], tor_u[:], tor_u[:], ALU.mult)
                N2 = torp.tile([P, NC, 7], f32, tag="n2")
                nc.gpsimd.tensor_tensor(
                    N2[:], SQ[:, :, :, 0], SQ[:, :, :, 1], ALU.add)
                NRM = torp.tile([P, NC, 7], f32, tag="nrm")
                nc.scalar.activation(NRM[:], N2[:], ACT.Sqrt,
                                     bias=TORSION_EPS, scale=1.0)
                REC = torp.tile([P, NC, 7], f32, tag="rec")
                nc.vector.reciprocal(REC[:], NRM[:])
                PN = torp.tile([P, NC, 7, 2], f32, tag="pn")
                nc.gpsimd.tensor_tensor(
                    PN[:], tor_u[:],
                    REC[:].unsqueeze(3).broadcast_to([P, NC, 7, 2]),
                    ALU.mult,
                )
                DV = []
                for name, TTRUE in (("t", TORT), ("a", TORA)):
                    DF = torp.tile([P, NC, 7, 2], f32, tag=f"df{name}")
                    nc.gpsimd.tensor_tensor(DF[:], TTRUE[:], PN[:],
                                            ALU.subtract)
                    DS = torp.tile([P, NC, 7, 2], f32, tag=f"ds{name}")
                    nc.gpsimd.tensor_tensor(DS[:], DF[:], DF[:], ALU.mult)
                    D2T = torp.tile([P, NC, 7], f32, tag=f"d2t{name}")
                    nc.gpsimd.tensor_tensor(
                        D2T[:], DS[:, :, :, 0], DS[:, :, :, 1], ALU.add)
                    DVt = torp.tile([P, NC, 7], f32, tag=f"dv{name}")
                    nc.scalar.activation(DVt[:], D2T[:], ACT.Sqrt,
                                         bias=TORSION_EPS, scale=1.0)
                    DV.append(DVt)
                MN = torp.tile([P, NC, 7], f32, tag="mn")
                nc.vector.tensor_tensor(MN[:], DV[0][:], DV[1][:], ALU.min)
                nc.vector.tensor_reduce(FIN[:, 2 + u:3 + u], MN[:], AX.XY,
                                        ALU.add)
                AN = torp.tile([P, NC, 7], f32, tag="an")
                nc.vector.tensor_scalar(AN[:], NRM[:], 1.0, None,
                                        ALU.subtract)
                nc.vector.tensor_reduce(
                    FIN[:, 4 + u:5 + u], AN[:], AX.XY, ALU.add,
                    apply_absolute_value=True,
                )


            emit_unit(0, [])
            emit_torsion(0)
            emit_unit(1, [])

            # ---- colsum wrap-up (mixed units is fine: same b, same scale)
            if total_cs_mms > 0:
                CSUM = persist.tile([P, 2], f32, tag="csum")
                nc.vector.tensor_reduce(CSUM[0:32, 0:1], CS[0:32, :], AX.X,
                                        ALU.add)
                if total_cs_mms > 1:
                    nc.vector.tensor_reduce(CSUM[0:32, 1:2], CS[64:96, :],
                                            AX.X, ALU.add)
                else:
                    nc.vector.memset(CSUM[0:32, 1:2], 0.0)
                nc.vector.tensor_tensor(FIN[0:32, 6:7], CSUM[0:32, 0:1],
                                        CSUM[0:32, 1:2], ALU.add)

            # ---- fape partials from ACC (DVE-route accum columns)
            for u in range(2):
                FSC = torp.tile([P, 1], f32, tag=f"fsc{u}", name=f"FSC{u}")
                nc.vector.tensor_reduce(
                    FSC[:], ACC[:, u * 40:(u + 1) * 40], AX.X, ALU.add)
                nc.vector.tensor_copy(FIN[:, u:u + 1], FSC[:])

            emit_torsion(1)

            # ---- cross-partition reduce via ones-matmul
            fin_ps = psA.tile([P, 2, 512], f32, tag="a")
            nc.tensor.matmul(
                fin_ps[0:1, 0, 0:8],
                lhsT=ONES[:],
                rhs=FIN[:],
                start=True, stop=True,
            )
            OUT = persist.tile([1, 8], f32, tag="out")
            nc.scalar.copy(OUT[:], fin_ps[0:1, 0, 0:8])
            nc.sync.dma_start(out_d.ap(), OUT[:])

    nc.compile()
    return nc


def pack_inputs(traj_rotations, traj_translations, traj_torsions,
                true_rotations, true_translations,
                true_torsion_angles, true_torsion_angles_alt):
    """Build the 8 per-core input maps (host-side shard + layout)."""

    def chunked(x):
        # [N, ...] -> [P, NC, ...]  with i = c*128 + p
        return np.ascontiguousarray(
            x.reshape(NC, P, *x.shape[1:]).transpose(1, 0, *range(2, x.ndim + 1))
        )

    npbf = mybir.dt.np(mybir.dt.bfloat16)
    in_maps = []
    for k in range(8):
        b = k // 4
        ls = [(2 * k) % 8, (2 * k) % 8 + 1]
        feats = np.zeros((2, 2, K, N), npbf)
        tor = np.zeros((P, 2, NC, 7, 2), np.float32)
        for u, l in enumerate(ls):
            # Mt rows: [Rp; -Rt; c], z = [t_p; t_t; 1]
            mt = np.empty((N, 7, 3), np.float32)
            mt[:, 0:3, :] = traj_rotations[l, b]
            mt[:, 3:6, :] = -true_rotations[b]
            zv = np.empty((N, 7), np.float32)
            zv[:, 0:3] = traj_translations[l, b]
            zv[:, 3:6] = true_translations[b]
            zv[:, 6] = 1.0
            mt[:, 6, :] = -np.einsum('nm,nmr->nr', zv[:, 0:6], mt[:, 0:6, :])
            F = np.einsum('nar,nbr->nab', mt, mt).reshape(N, K)
            Z = np.einsum('na,nb->nab', zv, zv).reshape(N, K)
            feats[u, 0] = F.T.astype(npbf)
            feats[u, 1] = Z.T.astype(npbf)
            tor[:, u] = chunked(traj_torsions[l, b])
        in_maps.append({
            "feats": feats,
            "tor": tor,
            "tort": chunked(true_torsion_angles[b]),
            "tora": chunked(true_torsion_angles_alt[b]),
        })
    return in_maps


def combine_outputs(results):
    """results: list of 8 dicts with 'out' [1,8] -> full output [B] f32."""
    total = np.zeros(B, np.float64)
    for k in range(8):
        b = k // 4
        o = results[k]["out"][0].astype(np.float64)
        for u in range(2):
            fape = o[u] / (N * N) / Z_SCALE
            tor = o[2 + u] / (7 * N) + 0.02 * o[4 + u] / (7 * N)
            total[b] += fape + tor
        # colsum partial (both units; 32 identical ones-matmul rows)
        total[b] += o[6] / 32.0 / (N * N) / Z_SCALE
    return (total / L).astype(np.float32)


def _install_ntff_shim():
    """The image's antenv lacks axon_hooks; synthesize it so trace=True can
    drive NTFF profiling via the ctypes hook in trn_agent_boot."""
    import types
    if "antenv.axon_hooks" in sys.modules:
        return
    try:
        from trn_agent_boot.trn_boot import _ntff_profile_via_ctypes
        hook = _ntff_profile_via_ctypes("/opt/axon/libaxon_pjrt.so")
    except Exception:
        hook = None
    mod = types.ModuleType("antenv.axon_hooks")
    mod._hook = hook
    mod.get_axon_ntff_profile_hook = lambda: mod._hook
    mod.set_axon_ntff_profile_hook = lambda h: setattr(mod, "_hook", h)
    sys.modules["antenv.axon_hooks"] = mod


def kernel(**inputs):
    if "nc" not in _cache:
        _cache["nc"] = build_program(
            float(os.environ.get("KERNEL_DVE_FRAC", "0.62")))
    nc = _cache["nc"]
    in_maps = pack_inputs(**{k: np.asarray(v) for k, v in inputs.items()})
    trace = bool(int(os.environ.get("KERNEL_TRACE", "0")))
    if trace:
        _install_ntff_shim()
    res = run_bass_kernel_spmd(
        nc, in_maps, list(range(8)),
        trace=trace,
    )
    _cache["last_results"] = res
    return combine_outputs(res.results)


# revision 7
# speedup vs baseline: 1.1538x; 1.0220x over previous
"""Trainium2 Bass kernel v2 for nn_AuxiliaryLoss (FAPE + torsion loss).

Math: d2(i,j) = <F_i, Z_j> rank-49 quadratic form (see pack_inputs);
fape partial = sum_ij min(sqrt(d2+eps), 10).  Only the GLOBAL sum per
core matters (both units share b), so jobs sum in any grouping.

Per core (2 (l,b) units):
  PE : 128 rank-49 bf16 matmuls [128,512] into a 6-bank PSUM ring
       (two [128,3,512] tiles), row-group alternation per i-chunk;
       stacked bf16 feature transposes into a reserved bank; ones-colsum
       of a fraction of jobs into the 8th bank (column-group alternation).
  ACT: sqrt(d2 + 0.25) f32->bf16, [128,3,512] jobs.  The 0.25 bias
       absorbs bf16 rounding of near-zero d2 so sqrt never sees negative
       inputs (error ~0.1%, validated).
  DVE: feature products (f32, 2x), bf16 unstack copies (2x), then per
       job either fused min+sum (CACHE_REDUCE) or min-only (4x) for the
       PE-colsum route.
  GPSIMD/ACT: torsion loss (identical to baseline).
"""
import os
import sys
import numpy as np

sys.path.insert(0, "/opt/trn_rl_repo")

import concourse.bacc as bacc
import concourse.tile as tile
import concourse.mybir as mybir
from concourse.bass_utils import run_bass_kernel_spmd

f32 = mybir.dt.float32
bf16 = mybir.dt.bfloat16
ACT = mybir.ActivationFunctionType
ALU = mybir.AluOpType
AX = mybir.AxisListType

L, B, N = 8, 2, 2048
NC = 16          # i-chunks of 128
P = 128
K = 49
D_CLAMP = 10.0
SQRT_BIAS = 0.25
Z_SCALE = 10.0
TORSION_EPS = 1e-8

JOB_WIDTHS = [2] * 32                  # 64 matmuls per unit
assert sum(JOB_WIDTHS) == 64

_cache = {}


def build_program(dve_frac=0.62):
    # route: True -> DVE fused min+sum; False -> DVE min + PE colsum.
    # Colsum jobs go at the END of each unit: early in the unit the DVE
    # still drains the feature-product chain, and a colsum matmul waiting
    # on DVE's min pass would block the in-order PE queue.
    # greedy split (the best-measured interleave), same for both units
    dve_route = []
    dcols, tot = 0.0, 0.0
    for w in JOB_WIDTHS:
        dve_route.append(dcols <= dve_frac * tot + 1e-9)
        if dve_route[-1]:
            dcols += w
        tot += w
    dve_route[-1] = True
    routes = [dve_route, dve_route]

    nc = bacc.Bacc("TRN2", target_bir_lowering=False, debug=False)

    def register_const_ap(value, dtype=f32):
        t = nc.alloc_sbuf_tensor(f"const-{dtype.name}-{value}", [128, 1], dtype)
        nc.gpsimd.memset(t.ap(), value)
        nc.const_aps.aps[(dtype, value)] = t.ap()

    register_const_ap(SQRT_BIAS)
    register_const_ap(TORSION_EPS)
    nc.all_engine_barrier()

    # ---- DRAM I/O (per core); feats = host-packed transposed F/Z
    feats_d = nc.dram_tensor("feats", [2, 2, K, N], bf16, kind="ExternalInput")
    tor_d = nc.dram_tensor("tor", [P, 2, NC, 7, 2], f32, kind="ExternalInput")
    tort_d = nc.dram_tensor("tort", [P, NC, 7, 2], f32, kind="ExternalInput")
    tora_d = nc.dram_tensor("tora", [P, NC, 7, 2], f32, kind="ExternalInput")
    out_d = nc.dram_tensor("out", [1, 8], f32, kind="ExternalOutput")

    with tile.TileContext(nc) as tc:
        import contextlib
        with contextlib.ExitStack() as ctx:
            persist = ctx.enter_context(tc.tile_pool(name="persist", bufs=1))
            feat = ctx.enter_context(tc.tile_pool(name="feat", bufs=1))
            sp = ctx.enter_context(tc.tile_pool(name="sp", bufs=12))
            msp = ctx.enter_context(tc.tile_pool(name="msp", bufs=12))
            torp = ctx.enter_context(tc.tile_pool(name="torp", bufs=2))
            psA = ctx.enter_context(tc.tile_pool(name="psA", bufs=3, space="PSUM"))
            psC = ctx.enter_context(tc.tile_pool(name="psC", bufs=1, space="PSUM"))

            # ---- persistent inputs (torsion loads emitted after the
            # feature loads below; they are needed much later)
            TOR = persist.tile([P, 2, NC, 7, 2], f32, tag="tor")
            TORT = persist.tile([P, NC, 7, 2], f32, tag="tort")
            TORA = persist.tile([P, NC, 7, 2], f32, tag="tora")

            ACC = persist.tile([P, 2 * 40], f32, tag="acc")
            nc.vector.memset(ACC[:], 0.0)
            FIN = persist.tile([P, 8], f32, tag="fin")
            nc.vector.memset(FIN[:], 0.0)
            ONES = persist.tile([P, 1], f32, tag="ones")
            nc.vector.memset(ONES[:], 1.0)
            OBF = persist.tile([P, 32], bf16, tag="obf")
            nc.vector.memset(OBF[:], 1.0)
            WRM = persist.tile([P, 1], f32, tag="wrm")
            nc.vector.memset(WRM[:], 1.0)
            nc.scalar.activation(WRM[:], WRM[:], ACT.Sqrt, bias=SQRT_BIAS,
                                 scale=1.0)

            # PSUM: 3-deep ring of 2-bank tiles, CS 1 bank, TT scratch
            CS = psC.tile([P, 512], f32, tag="cs")

            # ---- feature tensors: loaded pre-transposed from host
            FT = [[feat.tile([P, N], bf16, tag=f"ft{u}{t}",
                             name=f"FT{u}{t}")
                   for t in range(2)] for u in range(2)]
            FT2 = [[feat.tile([64 + K, N], bf16, tag=f"ft2{u}{t}",
                              name=f"FT2{u}{t}")
                    for t in range(2)] for u in range(2)]
            # unit-0's four tensors first, spread over all three queues
            loads = [(0, 0, False, nc.sync), (0, 0, True, nc.scalar),
                     (0, 1, False, nc.gpsimd), (0, 1, True, nc.sync),
                     (1, 0, False, nc.scalar), (1, 0, True, nc.gpsimd),
                     (1, 1, False, nc.sync), (1, 1, True, nc.scalar)]
            for u, t, dup, q in loads:
                dst = (FT2[u][t][64:64 + K, :] if dup
                       else FT[u][t][0:K, :])
                q.dma_start(dst, feats_d.ap()[u, t])
            nc.gpsimd.dma_start(TOR[:], tor_d.ap())
            nc.gpsimd.dma_start(TORT[:], tort_d.ap())
            nc.scalar.dma_start(TORA[:], tora_d.ap())

            # ---- main loop
            state = {"ncs": 0, "nacc": 0}
            total_cs_mms = sum(
                0 if r else w
                for route in routes for r, w in zip(route, JOB_WIDTHS))

            CS_LAG = 16

            def emit_unit(u, weave_ops):
                dve_route = routes[u]
                pending = []   # (MS tile, width) awaiting colsum emission

                def emit_colsum(MSp, wp):
                    for k in range(wp):
                        cg = 64 * (state["ncs"] % 2)
                        nc.tensor.matmul(
                            CS[cg:cg + 32, :],
                            lhsT=OBF[:],
                            rhs=MSp[:, k, :],
                            start=(state["ncs"] < 2),
                            stop=(state["ncs"] >= total_cs_mms - 2),
                            tile_position=(0, cg),
                            skip_group_check=True,
                        )
                        state["ncs"] += 1

                njob = 0
                mm = 0
                for j, w in enumerate(JOB_WIDTHS):
                    tile_t = psA.tile([P, 2, 512], f32, tag="a")
                    for k in range(w):
                        m = mm + k
                        c = m // 4
                        n = m % 4
                        rg = 64 * (m % 2)
                        lhs = (FT[u][0][0:K] if rg == 0
                               else FT2[u][0][64:64 + K])
                        rhs = (FT[u][1][0:K] if rg == 0
                               else FT2[u][1][64:64 + K])
                        nc.tensor.matmul(
                            tile_t[:, k, :],
                            lhsT=lhs[:, c * P:(c + 1) * P],
                            rhs=rhs[:, n * 512:(n + 1) * 512],
                            start=True, stop=True,
                            tile_position=(rg, 0),
                        )
                    mm += w
                    S = sp.tile([P, 2, 512], bf16, tag="s")
                    nc.scalar.activation(
                        S[:, 0:w, :], tile_t[:, 0:w, :], ACT.Sqrt,
                        bias=SQRT_BIAS, scale=1.0)
                    MS = msp.tile([P, 2, 512], bf16, tag="ms")
                    if dve_route[njob]:
                        nc.vector.tensor_scalar(
                            MS[:, 0:w, :], S[:, 0:w, :], D_CLAMP, None,
                            ALU.min, ALU.add,
                            accum_out=ACC[:, u * 40 + state["nacc"]:
                                          u * 40 + state["nacc"] + 1])
                        state["nacc"] += 1
                    else:
                        nc.vector.tensor_scalar(
                            MS[:, 0:w, :], S[:, 0:w, :], 0.0, D_CLAMP,
                            ALU.max, ALU.min)
                        pending.append((MS, w, njob))
                    while pending and pending[0][2] <= njob - CS_LAG:
                        MSp, wp, _ = pending.pop(0)
                        emit_colsum(MSp, wp)
                    njob += 1
                    if weave_ops and njob >= 15 and njob % 2 == 1:
                        weave_ops.pop(0)()
                for MSp, wp, _ in pending:
                    emit_colsum(MSp, wp)
                state["nacc"] = 0

            # ---- torsion loss per unit (emitted early for overlap)
            def emit_torsion(u):
                tor_u = TOR[:, u]  # [P, NC, 7, 2]
                SQ = torp.tile([P, NC, 7, 2], f32, tag="sq")
                nc.gpsimd.tensor_tensor(SQ[:], tor_u[:], tor_u[:], ALU.mult)
                N2 = torp.tile([P, NC, 7], f32, tag="n2")
                nc.gpsimd.tensor_tensor(
                    N2[:], SQ[:, :, :, 0], SQ[:, :, :, 1], ALU.add)
                NRM = torp.tile([P, NC, 7], f32, tag="nrm")
                nc.scalar.activation(NRM[:], N2[:], ACT.Sqrt,
                                     bias=TORSION_EPS, scale=1.0)
                REC = torp.tile([P, NC, 7], f32, tag="rec")
                nc.vector.reciprocal(REC[:], NRM[:])
                PN = torp.tile([P, NC, 7, 2], f32, tag="pn")
                nc.gpsimd.tensor_tensor(
                    PN[:], tor_u[:],
                    REC[:].unsqueeze(3).broadcast_to([P, NC, 7, 2]),
                    ALU.mult,
                )
                DV = []
                for name, TTRUE in (("t", TORT), ("a", TORA)):
                    DF = torp.tile([P, NC, 7, 2], f32, tag=f"df{name}")
                    nc.gpsimd.tensor_tensor(DF[:], TTRUE[:], PN[:],
                                            ALU.subtract)
                    DS = torp.tile([P, NC, 7, 2], f32, tag=f"ds{name}")
                    nc.gpsimd.tensor_tensor(DS[:], DF[:], DF[:], ALU.mult)
                    D2T = torp.tile([P, NC, 7], f32, tag=f"d2t{name}")
                    nc.gpsimd.tensor_tensor(
                        D2T[:], DS[:, :, :, 0], DS[:, :, :, 1], ALU.add)
                    DVt = torp.tile([P, NC, 7], f32, tag=f"dv{name}")
                    nc.scalar.activation(DVt[:], D2T[:], ACT.Sqrt,
                                         bias=TORSION_EPS, scale=1.0)
                    DV.append(DVt)
                MN = torp.tile([P, NC, 7], f32, tag="mn")
                nc.vector.tensor_tensor(MN[:], DV[0][:], DV[1][:], ALU.min)
                nc.vector.tensor_reduce(FIN[:, 2 + u:3 + u], MN[:], AX.XY,
                                        ALU.add)
                AN = torp.tile([P, NC, 7], f32, tag="an")
                nc.vector.tensor_scalar(AN[:], NRM[:], 1.0, None,
                                        ALU.subtract)
                nc.vector.tensor_reduce(
                    FIN[:, 4 + u:5 + u], AN[:], AX.XY, ALU.add,
                    apply_absolute_value=True,
                )


            emit_unit(0, [])
            emit_torsion(0)
            emit_unit(1, [])

            # ---- colsum wrap-up (mixed units is fine: same b, same scale)
            if total_cs_mms > 0:
                CSUM = persist.tile([P, 2], f32, tag="csum")
                nc.vector.tensor_reduce(CSUM[0:32, 0:1], CS[0:32, :], AX.X,
                                        ALU.add)
                if total_cs_mms > 1:
                    nc.vector.tensor_reduce(CSUM[0:32, 1:2], CS[64:96, :],
                                            AX.X, ALU.add)
                else:
                    nc.vector.memset(CSUM[0:32, 1:2], 0.0)
                nc.vector.tensor_tensor(FIN[0:32, 6:7], CSUM[0:32, 0:1],
                                        CSUM[0:32, 1:2], ALU.add)

            # ---- fape partials from ACC (DVE-route accum columns)
            for u in range(2):
                FSC = torp.tile([P, 1], f32, tag=f"fsc{u}", name=f"FSC{u}")
                nc.vector.tensor_reduce(
                    FSC[:], ACC[:, u * 40:(u + 1) * 40], AX.X, ALU.add)
                nc.vector.tensor_copy(FIN[:, u:u + 1], FSC[:])

            emit_torsion(1)

            # ---- cross-partition reduce via ones-matmul
            fin_ps = psA.tile([P, 2, 512], f32, tag="a")
            nc.tensor.matmul(
                fin_ps[0:1, 0, 0:8],
                lhsT=ONES[:],
                rhs=FIN[:],
                start=True, stop=True,
            )
            OUT = persist.tile([1, 8], f32, tag="out")
            nc.scalar.copy(OUT[:], fin_ps[0:1, 0, 0:8])
            nc.sync.dma_start(out_d.ap(), OUT[:])

    nc.compile()
    return nc


def pack_inputs(traj_rotations, traj_translations, traj_torsions,
                true_rotations, true_translations,
                true_torsion_angles, true_torsion_angles_alt):
    """Build the 8 per-core input maps (host-side shard + layout)."""

    def chunked(x):
        # [N, ...] -> [P, NC, ...]  with i = c*128 + p
        return np.ascontiguousarray(
            x.reshape(NC, P, *x.shape[1:]).transpose(1, 0, *range(2, x.ndim + 1))
        )

    npbf = mybir.dt.np(mybir.dt.bfloat16)
    in_maps = []
    for k in range(8):
        b = k // 4
        ls = [(2 * k) % 8, (2 * k) % 8 + 1]
        feats = np.zeros((2, 2, K, N), npbf)
        tor = np.zeros((P, 2, NC, 7, 2), np.float32)
        for u, l in enumerate(ls):
            # Mt rows: [Rp; -Rt; c], z = [t_p; t_t; 1]
            mt = np.empty((N, 7, 3), np.float32)
            mt[:, 0:3, :] = traj_rotations[l, b]
            mt[:, 3:6, :] = -true_rotations[b]
            zv = np.empty((N, 7), np.float32)
            zv[:, 0:3] = traj_translations[l, b]
            zv[:, 3:6] = true_translations[b]
            zv[:, 6] = 1.0
            mt[:, 6, :] = -np.einsum('nm,nmr->nr', zv[:, 0:6], mt[:, 0:6, :])
            F = np.einsum('nar,nbr->nab', mt, mt).reshape(N, K)
            Z = np.einsum('na,nb->nab', zv, zv).reshape(N, K)
            feats[u, 0] = F.T.astype(npbf)
            feats[u, 1] = Z.T.astype(npbf)
            tor[:, u] = chunked(traj_torsions[l, b])
        in_maps.append({
            "feats": feats,
            "tor": tor,
            "tort": chunked(true_torsion_angles[b]),
            "tora": chunked(true_torsion_angles_alt[b]),
        })
    return in_maps


def combine_outputs(results):
    """results: list of 8 dicts with 'out' [1,8] -> full output [B] f32."""
    total = np.zeros(B, np.float64)
    for k in range(8):
        b = k // 4
        o = results[k]["out"][0].astype(np.float64)
        for u in range(2):
            fape = o[u] / (N * N) / Z_SCALE
            tor = o[2 + u] / (7 * N) + 0.02 * o[4 + u] / (7 * N)
            total[b] += fape + tor
        # colsum partial (both units; 32 identical ones-matmul rows)
        total[b] += o[6] / 32.0 / (N * N) / Z_SCALE
    return (total / L).astype(np.float32)


def _install_ntff_shim():
    """The image's antenv lacks axon_hooks; synthesize it so trace=True can
    drive NTFF profiling via the ctypes hook in trn_agent_boot."""
    import types
    if "antenv.axon_hooks" in sys.modules:
        return
    try:
        from trn_agent_boot.trn_boot import _ntff_profile_via_ctypes
        hook = _ntff_profile_via_ctypes("/opt/axon/libaxon_pjrt.so")
    except Exception:
        hook = None
    mod = types.ModuleType("antenv.axon_hooks")
    mod._hook = hook
    mod.get_axon_ntff_profile_hook = lambda: mod._hook
    mod.set_axon_ntff_profile_hook = lambda h: setattr(mod, "_hook", h)
    sys.modules["antenv.axon_hooks"] = mod


def kernel(**inputs):
    if "nc" not in _cache:
        _cache["nc"] = build_program(
            float(os.environ.get("KERNEL_DVE_FRAC", "0.62")))
    nc = _cache["nc"]
    in_maps = pack_inputs(**{k: np.asarray(v) for k, v in inputs.items()})
    trace = bool(int(os.environ.get("KERNEL_TRACE", "0")))
    if trace:
        _install_ntff_shim()
    res = run_bass_kernel_spmd(
        nc, in_maps, list(range(8)),
        trace=trace,
    )
    _cache["last_results"] = res
    return combine_outputs(res.results)
